# revision 31
# baseline (speedup 1.0000x reference)
"""TRN2 Bass kernel for batched dot-product attention (no scale, eval mode).

reference:
    score   = einsum('bqd,bvd->bqv', query, value)      # B=16, L=2048, D=1024
    attn    = softmax(score, axis=-1)
    context = einsum('bqv,bvd->bqd', attn, value)

Sharding: data-parallel over batch; each of 8 NeuronCores handles 2 batch
elements, no communication. Inputs are pre-cast to fp16 on the host (part of
the sharding/layout prep); matmuls run fp16 with fp32 PSUM accumulation.

v9: the PE issues nothing but the 2048 real matmuls per core (the fp16
roofline); every layout transpose (Q, V, P) runs on the DMA engines via the
XBAR transpose path (dma_start_transpose). All DMA lives on the single sync
HWDGE queue: mixing HWDGE queues serializes them against each other (each
ring switch waits for the other ring's last DMA). The in-order ring is kept
ahead of the PE by prefetching QT two tiles ahead, deferring stores by one
tile, and staggering Vn/VT prefetch chunks one per iteration.

Per-core per-batch plan:
  - preload V: natural fp16 copy Vn (4 chunked DMAs) + transposed VT via one
    XBAR-transpose DMA per 512-row v-group (DRAM -> SBUF)
  - per 128-row q-tile (software-pipelined, tail lags one tile):
      head: QT via XBAR-transpose DMA (DRAM -> SBUF, prefetched 2 tiles
            ahead), S = QT.T @ VT -> PSUM, per-chunk rowmax on DVE
      tail: exp(S - max) on ACT (fp16 P, fused rowsum), P^T via one
            SBUF -> SBUF XBAR-transpose DMA, O = PT.T @ Vn -> PSUM,
            multiply by 1/rowsum, store (deferred one tile)
"""

from contextlib import ExitStack

import numpy as np

import concourse.tile as tile
from concourse import bacc, mybir
from concourse.bass_utils import run_bass_kernel_spmd

B, LQ, LV, D = 16, 2048, 2048, 1024
NCORES = 8
BPC = B // NCORES  # batches per core
P = 128
NQT = LQ // P  # 16 q tiles
NVT = LV // P  # 16 v tiles
ND = D // P  # 8 d tiles
VCH = 512  # MM1 moving-operand chunk (free dim)
NCH = LV // VCH  # 4
VTPC = VCH // P  # v-tiles per chunk (4)
DCH = 512  # MM2 moving-operand chunk
NDCH = D // DCH  # 2
NT = BPC * NQT  # total q tiles per core

f32 = mybir.dt.float32
f16 = mybir.dt.float16
f8 = mybir.dt.float8e4
DR = mybir.MatmulPerfMode.DoubleRow
EXP = mybir.ActivationFunctionType.Exp
AX = mybir.AxisListType.X


def build_nc():
    nc = bacc.Bacc("TRN2", target_bir_lowering=False, debug=False)
    # MM1 operands arrive transposed ([b, d, seq]) and split on the host into
    # fp8 value + fp8 residual (S = Q8 V8 + Q8 dV8 + dQ8 V8 recovers fp16-level
    # scores at fp8 DoubleRow speed). v16 stays natural fp16 for MM2.
    qt_d = nc.dram_tensor("qt8", [BPC, 2, D, LQ], f8, kind="ExternalInput").ap()
    v_d = nc.dram_tensor("v16", [BPC, LV, D], f16, kind="ExternalInput").ap()
    vt_d = nc.dram_tensor("vt8", [BPC, 2, D, LV], f8, kind="ExternalInput").ap()
    o_d = nc.dram_tensor("o", [BPC, LQ, D], f32, kind="ExternalOutput").ap()

    with tile.TileContext(nc) as tc, ExitStack() as ctx:
        vnpool = ctx.enter_context(tc.tile_pool(name="vnpool", bufs=2))
        vtpool = ctx.enter_context(tc.tile_pool(name="vtpool", bufs=2))
        qtp = ctx.enter_context(tc.tile_pool(name="qtp", bufs=3))
        pp = ctx.enter_context(tc.tile_pool(name="pp", bufs=3))
        ptp = ctx.enter_context(tc.tile_pool(name="ptp", bufs=3))
        outp = ctx.enter_context(tc.tile_pool(name="outp", bufs=3))
        statp = ctx.enter_context(tc.tile_pool(name="statp", bufs=2))
        psum = ctx.enter_context(tc.tile_pool(name="psum", bufs=4, space="PSUM"))

        def alloc_vn(b):
            return vnpool.tile([P, NVT, D], f16, tag="Vn", name=f"Vn{b}")

        def load_vn_chunk(b, Vn, g):
            """One quarter of V natural fp16: Vn[p, j, d] = V[128j + p, d]."""
            nc.sync.dma_start(
                out=Vn[:, g * VTPC : (g + 1) * VTPC, :],
                in_=v_d[b, g * VCH : (g + 1) * VCH].rearrange(
                    "(j p) d -> p j d", p=P
                ),
            )

        def alloc_vt(b):
            return vtpool.tile([P, 2, NCH, ND, VCH], f8, tag="VT", name=f"VT{b}")

        def load_vt_group(b, VT, g):
            """VT8 + residual from the host copy: two plain DMAs per 512-col
            v-group. VT[dmod, r, g, k, v] = VT8_host[r, 128k + dmod, 512g + v]."""
            for r in range(2):
                nc.sync.dma_start(
                    out=VT[:, r, g],
                    in_=vt_d[b, r, :, g * VCH : (g + 1) * VCH].rearrange(
                        "(k p) v -> p k v", p=P
                    ),
                )

        def load_qt(b, qg):
            """QT8 + residual for 4 q-tiles, two plain DMAs.
            QT[dmod, r, k, q] = QT8_host[r, 128k + dmod, 512qg + q]."""
            QT = qtp.tile([P, 2, ND, 4 * P], f8, tag="QT", name=f"QT{b}_{qg}")
            for r in range(2):
                nc.sync.dma_start(
                    out=QT[:, r],
                    in_=qt_d[b, r, :, qg * 4 * P : (qg + 1) * 4 * P].rearrange(
                        "(k p) q -> p k q", p=P
                    ),
                )
            return QT

        def head(b, qi, VT, QT):
            """S = Q8 V8 + Q8 dV8 + dQ8 V8 via fp8 DoubleRow matmuls (k-pairs),
            per-chunk row maxes. QT holds 4 q-tiles; this tile's stationary
            slices are QT[:, r, kp:kp+2, qs:qs+128]."""
            qs = (qi % 4) * P
            psS = [
                psum.tile([P, 2 * VCH], f32, tag="ps", name=f"psS{b}_{qi}_{h}")
                for h in range(NCH // 2)
            ]
            stats = statp.tile([P, NCH], f32, tag="stats", name=f"st{b}_{qi}")
            NKP = ND // 2
            terms = ((0, 0), (0, 1), (1, 0))  # (Q8,V8), (Q8,dV8), (dQ8,V8)
            for n in range(NCH):
                sl = slice((n % 2) * VCH, (n % 2 + 1) * VCH)
                for ti, (qr, vr) in enumerate(terms):
                    for kp in range(NKP):
                        nc.tensor.matmul(
                            psS[n // 2][:, sl],
                            QT[:, qr, 2 * kp : 2 * kp + 2, qs : qs + P],
                            VT[:, vr, n, 2 * kp : 2 * kp + 2, :],
                            start=(ti == 0 and kp == 0),
                            stop=(ti == len(terms) - 1 and kp == NKP - 1),
                            perf_mode=DR,
                        )
                nc.vector.reduce_max(stats[:, n : n + 1], psS[n // 2][:, sl], axis=AX)
            return psS, stats

        def softmax_part(b, qi, psS, stats):
            """Softmax on ACT/DVE: exp(S - max) -> fp16 P, plus 1/rowsum."""
            negmax = statp.tile([P, 1], f32, tag="negmax", name=f"nm{b}_{qi}")
            nc.vector.reduce_max(negmax, stats, axis=AX, negate=True)
            sums = statp.tile([P, NCH], f32, tag="sums", name=f"sm{b}_{qi}")
            Pt = pp.tile([P, LV], f16, tag="P", name=f"P{b}_{qi}")
            for n in range(NCH):
                sl = slice((n % 2) * VCH, (n % 2 + 1) * VCH)
                nc.scalar.activation(
                    Pt[:, n * VCH : (n + 1) * VCH],
                    psS[n // 2][:, sl],
                    EXP,
                    bias=negmax,
                    accum_out=sums[:, n : n + 1],
                )
            rowsum = statp.tile([P, 1], f32, tag="rowsum", name=f"rs{b}_{qi}")
            nc.vector.reduce_sum(rowsum, sums, axis=AX)
            rinv = statp.tile([P, 1], f32, tag="rinv", name=f"ri{b}_{qi}")
            nc.vector.reciprocal(rinv, rowsum)
            return Pt, rinv

        def transpose_p(b, qi, Pt):
            """P^T via one SBUF->SBUF XBAR-transpose DMA.
            PT[vmod, j, q] = P[q, 128j + vmod]."""
            PT = ptp.tile([P, NVT, P], f16, tag="PT", name=f"PT{b}_{qi}")
            nc.sync.dma_start_transpose(PT, Pt)
            return PT

        def tail(b, qi, PT, rinv, Vn):
            """O = P @ V, normalize. Store is deferred to the next iteration."""
            psO = psum.tile([P, D], f32, tag="ps", name=f"psO{b}_{qi}")
            out_sb = outp.tile([P, D], f32, tag="out", name=f"o{b}_{qi}")
            for dch in range(NDCH):
                sl = slice(dch * DCH, (dch + 1) * DCH)
                for j in range(NVT):
                    nc.tensor.matmul(
                        psO[:, sl],
                        PT[:, j, :],
                        Vn[:, j, sl],
                        start=(j == 0),
                        stop=(j == NVT - 1),
                    )
                nc.vector.tensor_scalar_mul(out_sb[:, sl], psO[:, sl], rinv)
            return out_sb

        def store(b, qi, out_sb):
            nc.sync.dma_start(o_d[b, qi * P : (qi + 1) * P, :], out_sb)

        NG = NT // 4  # q-tile groups of 4 (QT DMA granularity)
        pending = None  # tile whose softmax/MM2 still has to run
        done = None  # tile whose store still has to be issued
        # Startup: order the serial DMA chain by first use on the PE.
        qts = {0: load_qt(0, 0)}
        vt_next = alloc_vt(0)
        load_vt_group(0, vt_next, 0)
        qts[1] = load_qt(0, 1)
        for g in (1, 2, 3):
            load_vt_group(0, vt_next, g)
        vn_next = alloc_vn(0)
        for g in range(NCH):
            load_vn_chunk(0, vn_next, g)
        for t in range(NT):
            b, qi = divmod(t, NQT)
            if qi == 0:
                Vn, VT = vn_next, vt_next
            if pending is not None:
                pb, pq, ppsS, pstats, pVn = pending
                Ppt, prinv = softmax_part(pb, pq, ppsS, pstats)
            G = t // 4
            QT = qts[G]
            if t % 4 == 0:
                if G - 1 in qts:
                    del qts[G - 1]
                if G + 2 < NG:
                    gb, gg = divmod(G + 2, NQT // 4)
                    qts[G + 2] = load_qt(gb, gg)
            if pending is not None:
                PPT = transpose_p(pb, pq, Ppt)
            if done is not None:
                store(*done)
                done = None
            psS, stats = head(b, qi, VT, QT)
            if pending is not None:
                out_sb = tail(pb, pq, PPT, prinv, pVn)
                done = (pb, pq, out_sb)
            pending = (b, qi, psS, stats, Vn)
            if b + 1 < BPC and 5 <= qi <= 8:
                if qi == 5:
                    vn_next = alloc_vn(b + 1)
                load_vn_chunk(b + 1, vn_next, qi - 5)
            if b + 1 < BPC and 12 <= qi <= 15:
                if qi == 12:
                    vt_next = alloc_vt(b + 1)
                load_vt_group(b + 1, vt_next, qi - 12)
        pb, pq, ppsS, pstats, pVn = pending
        Ppt, prinv = softmax_part(pb, pq, ppsS, pstats)
        PPT = transpose_p(pb, pq, Ppt)
        if done is not None:
            store(*done)
        out_sb = tail(pb, pq, PPT, prinv, pVn)
        store(pb, pq, out_sb)

    nc.compile()
    return nc


_NC_CACHE = None


def _get_nc():
    global _NC_CACHE
    if _NC_CACHE is None:
        _NC_CACHE = build_nc()
    return _NC_CACHE


def kernel(query: np.ndarray, value: np.ndarray) -> np.ndarray:
    query = np.asarray(query)
    value = np.asarray(value)
    assert query.shape == (B, LQ, D) and value.shape == (B, LV, D)
    import ml_dtypes

    def split8(xt):
        """[B, D, L] fp32-ish -> [B, 2, D, L] fp8 (value, residual)."""
        x8 = xt.astype(ml_dtypes.float8_e4m3)
        d8 = (xt - x8.astype(np.float32)).astype(ml_dtypes.float8_e4m3)
        return np.ascontiguousarray(np.stack([x8, d8], axis=1))

    q16 = query.astype(np.float16)
    v16 = np.ascontiguousarray(value.astype(np.float16))
    qt8 = split8(q16.transpose(0, 2, 1).astype(np.float32))
    vt8 = split8(v16.transpose(0, 2, 1).astype(np.float32))
    nc = _get_nc()
    in_maps = [
        {
            "qt8": qt8[i * BPC : (i + 1) * BPC],
            "v16": v16[i * BPC : (i + 1) * BPC],
            "vt8": vt8[i * BPC : (i + 1) * BPC],
        }
        for i in range(NCORES)
    ]
    res = run_bass_kernel_spmd(nc, in_maps, list(range(NCORES)))
    out = np.concatenate([res.results[i]["o"] for i in range(NCORES)], axis=0)
    return out


# revision 36
# speedup vs baseline: 1.2027x; 1.2027x over previous
"""TRN2 Bass kernel for batched dot-product attention (no scale, eval mode).

reference:
    score   = einsum('bqd,bvd->bqv', query, value)      # B=16, L=2048, D=1024
    attn    = softmax(score, axis=-1)
    context = einsum('bqv,bvd->bqd', attn, value)

Sharding: data-parallel over batch; each of 8 NeuronCores handles 2 batch
elements, no communication. Inputs are pre-cast to fp16 on the host (part of
the sharding/layout prep); matmuls run fp16 with fp32 PSUM accumulation.

v9: the PE issues nothing but the 2048 real matmuls per core (the fp16
roofline); every layout transpose (Q, V, P) runs on the DMA engines via the
XBAR transpose path (dma_start_transpose). All DMA lives on the single sync
HWDGE queue: mixing HWDGE queues serializes them against each other (each
ring switch waits for the other ring's last DMA). The in-order ring is kept
ahead of the PE by prefetching QT two tiles ahead, deferring stores by one
tile, and staggering Vn/VT prefetch chunks one per iteration.

Per-core per-batch plan:
  - preload V: natural fp16 copy Vn (4 chunked DMAs) + transposed VT via one
    XBAR-transpose DMA per 512-row v-group (DRAM -> SBUF)
  - per 128-row q-tile (software-pipelined, tail lags one tile):
      head: QT via XBAR-transpose DMA (DRAM -> SBUF, prefetched 2 tiles
            ahead), S = QT.T @ VT -> PSUM, per-chunk rowmax on DVE
      tail: exp(S - max) on ACT (fp16 P, fused rowsum), P^T via one
            SBUF -> SBUF XBAR-transpose DMA, O = PT.T @ Vn -> PSUM,
            multiply by 1/rowsum, store (deferred one tile)
"""

from contextlib import ExitStack

import numpy as np

import concourse.tile as tile
from concourse import bacc, mybir
from concourse.bass_utils import run_bass_kernel_spmd

B, LQ, LV, D = 16, 2048, 2048, 1024
NCORES = 8
BPC = B // NCORES  # batches per core
P = 128
NQT = LQ // P  # 16 q tiles
NVT = LV // P  # 16 v tiles
ND = D // P  # 8 d tiles
VCH = 512  # MM1 moving-operand chunk (free dim)
NCH = LV // VCH  # 4
VTPC = VCH // P  # v-tiles per chunk (4)
DCH = 512  # MM2 moving-operand chunk
NDCH = D // DCH  # 2
NT = BPC * NQT  # total q tiles per core

f32 = mybir.dt.float32
f16 = mybir.dt.float16
EXP = mybir.ActivationFunctionType.Exp
AX = mybir.AxisListType.X


def build_nc():
    nc = bacc.Bacc("TRN2", target_bir_lowering=False, debug=False)
    # qt16/vt16 are transposed on the host ([b, d, seq]); v16 stays natural.
    qt_d = nc.dram_tensor("qt16", [BPC, D, LQ], f16, kind="ExternalInput").ap()
    v_d = nc.dram_tensor("v16", [BPC, LV, D], f16, kind="ExternalInput").ap()
    vt_d = nc.dram_tensor("vt16", [BPC, D, LV], f16, kind="ExternalInput").ap()
    o_d = nc.dram_tensor("o", [BPC, LQ, D], f32, kind="ExternalOutput").ap()

    with tile.TileContext(nc) as tc, ExitStack() as ctx:
        vnpool = ctx.enter_context(tc.tile_pool(name="vnpool", bufs=2))
        vtpool = ctx.enter_context(tc.tile_pool(name="vtpool", bufs=2))
        qtp = ctx.enter_context(tc.tile_pool(name="qtp", bufs=3))
        pp = ctx.enter_context(tc.tile_pool(name="pp", bufs=3))
        ptp = ctx.enter_context(tc.tile_pool(name="ptp", bufs=3))
        outp = ctx.enter_context(tc.tile_pool(name="outp", bufs=3))
        statp = ctx.enter_context(tc.tile_pool(name="statp", bufs=2))
        psum = ctx.enter_context(tc.tile_pool(name="psum", bufs=4, space="PSUM"))

        def alloc_vn(b):
            return vnpool.tile([P, NVT, D], f16, tag="Vn", name=f"Vn{b}")

        def load_vn_chunk(b, Vn, g):
            """One quarter of V natural fp16: Vn[p, j, d] = V[128j + p, d]."""
            nc.sync.dma_start(
                out=Vn[:, g * VTPC : (g + 1) * VTPC, :],
                in_=v_d[b, g * VCH : (g + 1) * VCH].rearrange(
                    "(j p) d -> p j d", p=P
                ),
            )

        def alloc_vt(b):
            return vtpool.tile([P, NCH, ND, VCH], f16, tag="VT", name=f"VT{b}")

        def load_vt_group(b, VT, g):
            """VT from the host-transposed copy: one plain DMA per 512-col
            v-group. VT[dmod, g, k, v] = VT_host[128k + dmod, 512g + v]."""
            nc.sync.dma_start(
                out=VT[:, g],
                in_=vt_d[b, :, g * VCH : (g + 1) * VCH].rearrange(
                    "(k p) v -> p k v", p=P
                ),
            )

        def load_qt(b, qg):
            """QT for 4 q-tiles from the host-transposed copy, one plain DMA.
            QT[dmod, k, q] = QT_host[128k + dmod, 512qg + q]."""
            QT = qtp.tile([P, ND, 4 * P], f16, tag="QT", name=f"QT{b}_{qg}")
            nc.sync.dma_start(
                out=QT,
                in_=qt_d[b, :, qg * 4 * P : (qg + 1) * 4 * P].rearrange(
                    "(k p) q -> p k q", p=P
                ),
            )
            return QT

        def head(b, qi, VT, QT):
            """S = Q @ V^T, per-chunk row maxes. QT holds 4 q-tiles; this
            tile's stationary slices are QT[:, k, qs:qs+128]."""
            qs = (qi % 4) * P
            psS = [
                psum.tile([P, 2 * VCH], f32, tag="ps", name=f"psS{b}_{qi}_{h}")
                for h in range(NCH // 2)
            ]
            stats = statp.tile([P, NCH], f32, tag="stats", name=f"st{b}_{qi}")
            for n in range(NCH):
                sl = slice((n % 2) * VCH, (n % 2 + 1) * VCH)
                for k in range(ND):
                    nc.tensor.matmul(
                        psS[n // 2][:, sl],
                        QT[:, k, qs : qs + P],
                        VT[:, n, k, :],
                        start=(k == 0),
                        stop=(k == ND - 1),
                    )
                nc.vector.reduce_max(stats[:, n : n + 1], psS[n // 2][:, sl], axis=AX)
            return psS, stats

        def softmax_part(b, qi, psS, stats):
            """Softmax on ACT/DVE: exp(S - max) -> fp16 P, plus 1/rowsum.
            The P^T XBAR transpose is split in halves so the first half can
            fly as soon as exp chunks 0-1 are done."""
            negmax = statp.tile([P, 1], f32, tag="negmax", name=f"nm{b}_{qi}")
            nc.vector.reduce_max(negmax, stats, axis=AX, negate=True)
            sums = statp.tile([P, NCH], f32, tag="sums", name=f"sm{b}_{qi}")
            Pt = pp.tile([P, LV], f16, tag="P", name=f"P{b}_{qi}")
            PT = ptp.tile([P, NVT, P], f16, tag="PT", name=f"PT{b}_{qi}")
            for n in range(NCH):
                sl = slice((n % 2) * VCH, (n % 2 + 1) * VCH)
                nc.scalar.activation(
                    Pt[:, n * VCH : (n + 1) * VCH],
                    psS[n // 2][:, sl],
                    EXP,
                    bias=negmax,
                    accum_out=sums[:, n : n + 1],
                )
                if n % 2 == 1:
                    # PT[vmod, j, q] = P[q, 128j + vmod] for this half
                    hf = n // 2
                    nc.sync.dma_start_transpose(
                        PT[:, hf * (NVT // 2) : (hf + 1) * (NVT // 2)],
                        Pt[:, hf * LV // 2 : (hf + 1) * LV // 2],
                    )
            rowsum = statp.tile([P, 1], f32, tag="rowsum", name=f"rs{b}_{qi}")
            nc.vector.reduce_sum(rowsum, sums, axis=AX)
            rinv = statp.tile([P, 1], f32, tag="rinv", name=f"ri{b}_{qi}")
            nc.vector.reciprocal(rinv, rowsum)
            return PT, rinv

        def tail(b, qi, PT, rinv, Vn):
            """O = P @ V, normalize. Store is deferred to the next iteration.
            j runs half-outer so the first PT half is consumed first."""
            psO = psum.tile([P, D], f32, tag="ps", name=f"psO{b}_{qi}")
            out_sb = outp.tile([P, D], f32, tag="out", name=f"o{b}_{qi}")
            JH = NVT // 2
            for hf in range(2):
                for dch in range(NDCH):
                    sl = slice(dch * DCH, (dch + 1) * DCH)
                    for j in range(hf * JH, (hf + 1) * JH):
                        nc.tensor.matmul(
                            psO[:, sl],
                            PT[:, j, :],
                            Vn[:, j, sl],
                            start=(j == 0),
                            stop=(j == NVT - 1),
                        )
            for dch in range(NDCH):
                sl = slice(dch * DCH, (dch + 1) * DCH)
                nc.vector.tensor_scalar_mul(out_sb[:, sl], psO[:, sl], rinv)
            return out_sb

        def store(b, qi, out_sb):
            nc.sync.dma_start(o_d[b, qi * P : (qi + 1) * P, :], out_sb)

        NG = NT // 4  # q-tile groups of 4 (QT DMA granularity)
        pending = None  # tile whose softmax/MM2 still has to run
        done = None  # tile whose store still has to be issued
        # Startup: order the serial DMA chain by first use on the PE.
        qts = {0: load_qt(0, 0)}
        vt_next = alloc_vt(0)
        load_vt_group(0, vt_next, 0)
        qts[1] = load_qt(0, 1)
        for g in (1, 2, 3):
            load_vt_group(0, vt_next, g)
        vn_next = alloc_vn(0)
        for g in range(NCH):
            load_vn_chunk(0, vn_next, g)
        for t in range(NT):
            b, qi = divmod(t, NQT)
            if qi == 0:
                Vn, VT = vn_next, vt_next
            if pending is not None:
                pb, pq, ppsS, pstats, pVn = pending
                PPT, prinv = softmax_part(pb, pq, ppsS, pstats)
            G = t // 4
            QT = qts[G]
            if t % 4 == 0:
                if G - 1 in qts:
                    del qts[G - 1]
                if G + 2 < NG:
                    gb, gg = divmod(G + 2, NQT // 4)
                    qts[G + 2] = load_qt(gb, gg)
            if done is not None:
                store(*done)
                done = None
            psS, stats = head(b, qi, VT, QT)
            if pending is not None:
                out_sb = tail(pb, pq, PPT, prinv, pVn)
                done = (pb, pq, out_sb)
            pending = (b, qi, psS, stats, Vn)
            if b + 1 < BPC and 5 <= qi <= 8:
                if qi == 5:
                    vn_next = alloc_vn(b + 1)
                load_vn_chunk(b + 1, vn_next, qi - 5)
            if b + 1 < BPC and 12 <= qi <= 15:
                if qi == 12:
                    vt_next = alloc_vt(b + 1)
                load_vt_group(b + 1, vt_next, qi - 12)
        pb, pq, ppsS, pstats, pVn = pending
        PPT, prinv = softmax_part(pb, pq, ppsS, pstats)
        if done is not None:
            store(*done)
        out_sb = tail(pb, pq, PPT, prinv, pVn)
        store(pb, pq, out_sb)

    nc.compile()
    return nc


_NC_CACHE = None


def _get_nc():
    global _NC_CACHE
    if _NC_CACHE is None:
        _NC_CACHE = build_nc()
    return _NC_CACHE


def kernel(query: np.ndarray, value: np.ndarray) -> np.ndarray:
    query = np.asarray(query)
    value = np.asarray(value)
    assert query.shape == (B, LQ, D) and value.shape == (B, LV, D)
    q16 = query.astype(np.float16)
    v16 = np.ascontiguousarray(value.astype(np.float16))
    qt16 = np.ascontiguousarray(q16.transpose(0, 2, 1))
    vt16 = np.ascontiguousarray(v16.transpose(0, 2, 1))
    nc = _get_nc()
    in_maps = [
        {
            "qt16": qt16[i * BPC : (i + 1) * BPC],
            "v16": v16[i * BPC : (i + 1) * BPC],
            "vt16": vt16[i * BPC : (i + 1) * BPC],
        }
        for i in range(NCORES)
    ]
    res = run_bass_kernel_spmd(nc, in_maps, list(range(NCORES)))
    out = np.concatenate([res.results[i]["o"] for i in range(NCORES)], axis=0)
    return out
